# revision 29
# baseline (speedup 1.0000x reference)
"""Trainium2 Bass kernel for AddNorm+1x1Conv+ReLU (fp16 pipeline, v2).

Computes: relu(einsum('bchw,oc->bohw', LN(x+y, axis=-1)*g + b, Wc))
with B=4, C=256, H=256, W=256, O=256.

Sharding: data-parallel over (B, H): core i handles b = i//2 and the
h-half i%2, i.e. an x/y shard of [C=256, 128, W=256].

Math folding:
  out[o,w] = relu(g[w]*M0[o,w] + Wsum[o]*b[w])        M0 = Wc @ z
           = g[w] * relu(M0[o,w] + Wsum[o]*(b/g)[w])  (g > 0)
  The rank-1 term Wsum x (b/g) is added in PSUM via a K=1 matmul; the
  g[w] rescale rides on the host-side unshard (diagonal per-w scale).

Device pipeline per group of H_GRP=8 h-rows (fp16 data, fp32 stats):
  DMA    : xg, yg [128, 2, 8, 256] f16
  DVE    : ag = xg + yg (one big tensor_tensor, 2x fp16 mode)
  DVE    : bn_stats on 2 rows interleaved [128, 256w, 2row] -> the even/odd
           lane stats ARE the two rows' (count, mean, count*var)
  ACT    : std = sqrt(cv/256 + eps)
  DVE    : rstd = 1/std;  nmr = -mean*rstd
  ACT/GpSimd (split per row): z = a*rstd + nmr   [activation(Identity,
           scale, bias) / tensor_scalar]
  PE     : psum[o, 2W] += WcT.T @ z, weight-stationary order (each lhsT
           streams all 4 banks of its ot half before switching)
  epilogue, split by ot half to balance PE vs DVE:
    ot=0 : rank-1 bias matmul on PE, then ACT relu-drain (2 banks/instr)
    ot=1 : DVE drain-add of precomputed r1 = Wsum x (b/g), then one big
           4x fp16 DVE relu
  DMA    : q out
Host: out = q.astype(f32) * g[w]  (relu(g*M + b*Wsum) = g*relu(M + (b/g)*Wsum), g>0).
"""

import numpy as np

import concourse.bass as bass
import concourse.tile as tile
from concourse import mybir
from concourse.bass_utils import run_bass_kernel_spmd

B, C, H, W, O = 4, 256, 256, 256, 256
N_CORES = 8
H_SHARD = (B * H) // N_CORES  # 128 h-rows per core
EPS = 1e-5

F16 = mybir.dt.float16
F32 = mybir.dt.float32
ALU = mybir.AluOpType
ACTFN = mybir.ActivationFunctionType


def _z_engine(r):
    """Row index r in 0..15 -> engine for the z pass (per group).
    ACT: 6 rows, GpSimd: 10 rows."""
    if r in (1, 3, 5, 7, 9, 11):
        return "act"
    return "gps"


def _bn_stats_raw(nc, out_ap, in_ap):
    """Emit InstBNStats directly: one 6-tuple out per partition, input is a
    512-element sequence. The Python wrapper's shape assert rejects the
    interleaved [128, w, 2] AP this kernel uses."""
    eng = nc.vector
    return eng.add_instruction(
        mybir.InstBNStats(
            name=nc.get_next_instruction_name(),
            ins=[eng.lower_ap(in_ap)],
            outs=[eng.lower_ap(out_ap)],
        )
    )


def build_graph(h_shard=H_SHARD, h_grp=8, split_waits=True):
    """One SPMD graph; every core runs it on its own shard."""
    assert h_shard % h_grp == 0 and h_grp % 4 == 0
    n_groups = h_shard // h_grp
    n_q = h_grp // 4  # psum tiles per ot (each = 2 h-pairs = 4 rows)

    nc = bass.Bass(trn_type="TRN2", target_bir_lowering=False)

    x_ext = nc.declare_dram_parameter("x", [C, h_shard, W], F16, isOutput=False)
    y_ext = nc.declare_dram_parameter("y", [C, h_shard, W], F16, isOutput=False)
    # wct[cin, ct, o] = Wc[o, ct*128+cin]  (lhsT layout, 2 c-tiles)
    wct_ext = nc.declare_dram_parameter("wct", [128, 2, O], F16, isOutput=False)
    # wsum[0, o] = sum_c Wc[o, c]
    wsum_ext = nc.declare_dram_parameter("wsum", [1, O], F16, isOutput=False)
    # bgbg[0, :] = concat(b/g, b/g)
    bgbg_ext = nc.declare_dram_parameter("bgbg", [1, 2 * W], F16, isOutput=False)
    out_ext = nc.declare_dram_parameter("out", [O, h_shard, W], F16, isOutput=True)

    # view [C, h, w] as [cin, ct, h, w] so one DMA covers both c-halves
    x_ap = x_ext.ap().rearrange("(t c) h w -> c t h w", t=2)
    y_ap = y_ext.ap().rearrange("(t c) h w -> c t h w", t=2)
    out_ap = out_ext.ap().rearrange("(t o) h w -> o t h w", t=2)

    with tile.TileContext(nc) as tc:
        from contextlib import ExitStack

        with ExitStack() as ctx:
            singles = ctx.enter_context(tc.tile_pool(name="singles", bufs=1))
            loads = ctx.enter_context(tc.tile_pool(name="loads", bufs=5))
            apool = ctx.enter_context(tc.tile_pool(name="apool", bufs=2))
            zpool = ctx.enter_context(tc.tile_pool(name="zpool", bufs=2))
            qpool = ctx.enter_context(tc.tile_pool(name="qpool", bufs=3))
            bnpool = ctx.enter_context(tc.tile_pool(name="bnpool", bufs=3))
            stats = ctx.enter_context(tc.tile_pool(name="stats", bufs=3))
            psum = ctx.enter_context(tc.tile_pool(name="psum", bufs=4, space="PSUM"))

            wct_sb = singles.tile([128, 2, O], F16, tag="wct")
            nc.gpsimd.dma_start(out=wct_sb[:], in_=wct_ext.ap())
            wsum_sb = singles.tile([1, O], F16, tag="wsum")
            nc.gpsimd.dma_start(out=wsum_sb[:], in_=wsum_ext.ap())
            bgbg_sb = singles.tile([1, 2 * W], F16, tag="bgbg")
            nc.gpsimd.dma_start(out=bgbg_sb[:], in_=bgbg_ext.ap())
            eps_sb = singles.tile([128, 1], F32, tag="eps")
            nc.vector.memset(eps_sb[:], EPS)

            inv256 = 1.0 / 256.0

            # r1[o, w'] = Wsum[o] * (b/g)[w'] for the ot=1 half: computed
            # once on PE, duplicated per psum bank for the DVE drain-add.
            r1pt = psum.tile([128, 2, 2 * W], F32, tag="pt")
            nc.tensor.matmul(
                r1pt[:, 1, :],
                lhsT=wsum_sb[0:1, 128:256],
                rhs=bgbg_sb[0:1, :],
                start=True, stop=True,
            )
            r1b = singles.tile([128, 2, 2 * W], F32, tag="r1b")
            for dup in range(2):
                nc.vector.tensor_copy(r1b[:, dup, :], r1pt[:, 1, :])

            for gi in range(n_groups):
                h0 = gi * h_grp

                xg = loads.tile([128, 2, h_grp, W], F16, tag="xg")
                yg = loads.tile([128, 2, h_grp, W], F16, tag="yg")
                nc.gpsimd.dma_start(out=xg[:], in_=x_ap[:, :, h0 : h0 + h_grp, :])
                nc.gpsimd.dma_start(out=yg[:], in_=y_ap[:, :, h0 : h0 + h_grp, :])

                # a = x + y, one big fp16 op (2x mode)
                ag = apool.tile([128, 2, h_grp, W], F16, tag="ag")
                nc.vector.tensor_tensor(
                    out=ag[:], in0=xg[:], in1=yg[:], op=ALU.add
                )

                # per-row stats: one bn_stats per 2 rows, input interleaved
                # [128, 256w, 2row] so even lanes = row0, odd lanes = row1.
                # 6-tuple out = (256, mean_r0, 256*var_r0, 256, mean_r1,
                # 256*var_r1).
                npair = h_grp // 2
                bnt = bnpool.tile([128, 2, npair, 6], F32, tag="bnt")
                for ct in range(2):
                    for jp in range(npair):
                        _bn_stats_raw(
                            nc,
                            bnt[:, ct, jp],
                            ag[:, ct, 2 * jp : 2 * jp + 2, :].rearrange(
                                "p r w -> p w r"
                            ),
                        )

                sshape = [128, 2, npair, 2]
                means = bnt[:, :, :, 1:5:3]  # [128, 2, npair, 2]
                cvs = bnt[:, :, :, 2:6:3]
                std = stats.tile(sshape, F32, tag="std")
                nc.scalar.activation(
                    out=std[:], in_=cvs, func=ACTFN.Sqrt,
                    bias=eps_sb[:], scale=inv256,
                )
                rstd = stats.tile(sshape, F32, tag="rstd")
                nc.vector.reciprocal(out=rstd[:], in_=std[:])
                # nmr = -mean*rstd
                nmr = stats.tile(sshape, F32, tag="nmr")
                nc.vector.scalar_tensor_tensor(
                    out=nmr[:], in0=means, scalar=-1.0, in1=rstd[:],
                    op0=ALU.mult, op1=ALU.mult,
                )

                # z = a*rstd + nmr, rows split across DVE / ACT / GpSimd
                zg = zpool.tile([128, 2, h_grp, W], F16, tag="zg")
                for ct in range(2):
                    for j in range(h_grp):
                        r = ct * h_grp + j
                        rs = rstd[:, ct, j // 2, j % 2 : j % 2 + 1]
                        nm = nmr[:, ct, j // 2, j % 2 : j % 2 + 1]
                        eng = _z_engine(r)
                        if eng == "act":
                            nc.scalar.activation(
                                out=zg[:, ct, j], in_=ag[:, ct, j],
                                func=ACTFN.Identity, scale=rs, bias=nm,
                            )
                        else:
                            e = nc.vector if eng == "dve" else nc.gpsimd
                            e.tensor_scalar(
                                out=zg[:, ct, j], in0=ag[:, ct, j],
                                scalar1=rs, scalar2=nm,
                                op0=ALU.mult, op1=ALU.add,
                            )

                # weight-stationary matmuls: per ot, each lhsT streams all
                # 2*n_q banks before switching (amortizes LDWEIGHTS).
                # ot=0: rank-1 bias on PE, relu-drain on ACT.
                # ot=1: bias added by the DVE drain, relu as one big DVE op.
                qg = qpool.tile([128, 2, n_q, 2, 2 * W], F16, tag="qg")
                for ot in range(2):
                    osl = slice(ot * 128, (ot + 1) * 128)
                    pts = []
                    for _q in range(n_q):
                        pt = psum.tile([128, 2, 2 * W], F32, tag="pt", name=f"pt{_q}")
                        pts.append(pt)
                    for ct in range(2):
                        for q in range(n_q):
                            for pi in range(2):
                                p = 2 * q + pi
                                hs = slice(2 * p, 2 * p + 2)
                                nc.tensor.matmul(
                                    pts[q][:, pi, :],
                                    lhsT=wct_sb[:, ct, osl],
                                    rhs=zg[:, ct, hs, :],
                                    start=(ct == 0),
                                    stop=(ct == 1 and ot == 1),
                                    skip_group_check=True,
                                )
                    if ot == 0:
                        for q in range(n_q):
                            for pi in range(2):
                                nc.tensor.matmul(
                                    pts[q][:, pi, :],
                                    lhsT=wsum_sb[0:1, 0:128],
                                    rhs=bgbg_sb[0:1, :],
                                    start=False, stop=True,
                                    skip_group_check=True,
                                )
                        for q in range(n_q):
                            nc.scalar.activation(
                                out=qg[:, 0, q], in_=pts[q][:], func=ACTFN.Relu,
                            )
                    else:
                        for q in range(n_q):
                            nc.vector.scalar_tensor_tensor(
                                out=qg[:, 1, q], in0=pts[q][:],
                                scalar=0.0, in1=r1b[:],
                                op0=ALU.add, op1=ALU.add,
                            )
                        nc.vector.tensor_scalar_max(
                            out=qg[:, 1], in0=qg[:, 1], scalar1=0.0
                        )

                # qg free layout = (ot, q, pi, 2 rows x 256 w) == (t, h, w)
                nc.sync.dma_start(
                    out=out_ap[:, :, h0 : h0 + h_grp, :], in_=qg[:]
                )

    if split_waits:
        _split_multiwaits(nc)
    return nc


def _split_multiwaits(nc):
    """This walrus build encodes at most one sync-wait per instruction.
    Hoist extra waits onto NoOp instructions inserted just before, on the
    same engine (same-engine stream order is preserved within the block)."""
    k = 0
    for f in nc.m.functions:
        for b in f.blocks:
            out = []
            for inst in b.instructions:
                si = getattr(inst, "sync_info", None)
                if si is not None and si.on_wait and len(si.on_wait) > 1:
                    waits = list(si.on_wait)
                    for w in waits[:-1]:
                        nop = mybir.InstNoOp(name=f"waitnop-{k}")
                        k += 1
                        nop.engine = inst.engine
                        nop.sync_info = mybir.SyncInfo(on_wait=[w], on_update=[])
                        out.append(nop)
                    inst.sync_info = mybir.SyncInfo(
                        on_wait=[waits[-1]], on_update=list(si.on_update or [])
                    )
                out.append(inst)
            b.instructions = out


def _shard_inputs(x, y, ln_weight, ln_bias, conv_weight, h_shard=H_SHARD):
    """Host-side preprocessing: per-core in_maps (fp16)."""
    g = np.asarray(ln_weight, np.float32)
    b = np.asarray(ln_bias, np.float32)
    wc = np.asarray(conv_weight, np.float32)  # [O, C]
    assert np.all(g > 0), "fast epilogue requires ln_weight > 0"

    wct = np.ascontiguousarray(
        wc.T.reshape(2, 128, O).transpose(1, 0, 2)
    ).astype(np.float16)  # [128, 2, O]
    wsum = wc.sum(axis=1)[None, :].astype(np.float16)  # [1, O]
    bg = (b / g).astype(np.float32)
    bgbg = np.concatenate([bg, bg])[None, :].astype(np.float16)  # [1, 2W]

    x16 = np.asarray(x, np.float16)
    y16 = np.asarray(y, np.float16)

    in_maps = []
    for i in range(N_CORES):
        bi, half = divmod(i, N_CORES // B)
        h0 = half * h_shard
        in_maps.append(
            {
                "x": np.ascontiguousarray(x16[bi, :, h0 : h0 + h_shard, :]),
                "y": np.ascontiguousarray(y16[bi, :, h0 : h0 + h_shard, :]),
                "wct": wct,
                "wsum": wsum,
                "bgbg": bgbg,
            }
        )
    return in_maps


_GRAPH = None


def _run(x, y, ln_weight, ln_bias, conv_weight, **spmd_kwargs):
    global _GRAPH
    if _GRAPH is None:
        _GRAPH = build_graph()
    in_maps = _shard_inputs(x, y, ln_weight, ln_bias, conv_weight)
    res = run_bass_kernel_spmd(
        _GRAPH, in_maps, core_ids=list(range(N_CORES)), **spmd_kwargs
    )
    g = np.asarray(ln_weight, np.float32)
    out = np.empty((B, O, H, W), np.float32)
    for i in range(N_CORES):
        bi, half = divmod(i, N_CORES // B)
        out[bi, :, half * H_SHARD : (half + 1) * H_SHARD, :] = np.asarray(
            res.results[i]["out"]
        ).reshape(O, H_SHARD, W)
    out *= g[None, None, None, :]
    return out, res


def kernel(x, y, ln_weight, ln_bias, conv_weight):
    out, _ = _run(x, y, ln_weight, ln_bias, conv_weight)
    return out
